# revision 30
# baseline (speedup 1.0000x reference)
"""Trainium2 Bass kernel for causal multi-head attention (AbstractNaiveMHA).

Problem shapes (hardcoded per the harness contract):
  x: [B=2, S=2048, D=1024] f32, mask: [B, S] int32 (all ones)
  Wq/Wk/Wv: [H=16, 64, 1024], bq/bk/bv: [16, 64], Wo: [1024, 1024], bo: [1024]

Sharding over 8 NeuronCores: core c -> batch b = c // 4, head group
g = c % 4 (heads 4g..4g+3).  Each core computes its 4 heads' attention
and a partial output projection through its column slice of Wo; the
host sums the 4 partials per batch and adds bo (the "all-reduce").

v3 design notes (all matmuls bf16, rel tol 2e-2 permits it):
  - t-chunked pipeline: for each 512-token i-chunk t: project q/k/v for
    chunk t, run attention for chunk t against j-tiles 0..4t+3, then the
    output projection for chunk t.  Fillers (proj t+1, out-proj t-1)
    overlap the ACT-engine exp.
  - Input DMA split: weights first, then xT per 512-chunk, so the first
    projection matmuls start ~5us in instead of waiting for the full 4MB.
  - Scores computed transposed (S^T[j,i]) with the two heads of a pair in
    PE row-quadrants 0:63/64:127 (quadrant matmuls dual-issue).
  - V path: vT computed v-major, then dma_start_transpose produces the
    token-major [V_even |ones| V_odd] tiles the AV matmul needs.
  - Causal: diagonal j-tiles restrict scores+exp+AV to i >= 128r, so the
    stale region of the es ring is never read (no pre-zeroing, no
    zero-mask); only the 128-wide intra-tile triangle is masked.
  - Softmax denominator rides free in the AV matmul via ones columns:
    av[0]=[out_e|den_e], av[1]=[den_o|out_o]; normalization uses
    cross-partition copies + reciprocal_approx_fast + aligned muls.
  - proj/out-proj PSUM groups share one double-buffered pool so the
    bias-add / PSUM-evacuation of group g drains under group g+1's
    matmuls.
  - Last chunk's tail: output-proj copies alternate ACT/DVE and the yT
    DMA goes out per 128-row tile to shrink the exposed epilogue.
"""

import os

import numpy as np
import ml_dtypes

import concourse.bass as bass
import concourse.mybir as mybir
import concourse.tile as tile
from concourse import bacc
from concourse.bass import ts, ds
from concourse.bass_utils import run_bass_kernel_spmd

B, S, D = 2, 2048, 1024
H, KD = 16, 64
P = 128
NT = S // 512      # 4 i-chunks of 512
NJ = S // P        # 16 j-tiles of 128
DCH = D // P       # 8 contraction chunks for the projections
N_CORES = 8
HEADS_PER_CORE = 4

F32 = mybir.dt.float32
BF16 = mybir.dt.bfloat16
EXP_SCALE = 1.0 / float(np.sqrt(np.float32(KD)))
KABL = set(os.environ.get("KABL", "").split(","))


def build_module(repeat: int = 1):
    """Build the single-core Bass module (same program on all 8 cores)."""
    nc = bacc.Bacc("TRN2", target_bir_lowering=False, debug=False)

    # all inputs arrive pre-swizzled into the SBUF layout (partition dim
    # first) so every cache DMA is a max-rate contiguous transfer
    xT = nc.dram_tensor("xT", [P, DCH, S], BF16, kind="ExternalInput").ap()
    wq = nc.dram_tensor("wq", [P, 2, DCH, P], BF16,
                        kind="ExternalInput").ap()
    wk = nc.dram_tensor("wk", [P, 2, DCH, P], BF16,
                        kind="ExternalInput").ap()
    wv = nc.dram_tensor("wv", [P, DCH, 2 * P], BF16,
                        kind="ExternalInput").ap()
    bqk = nc.dram_tensor("bqk", [P, 2, 2], F32, kind="ExternalInput").ap()
    bvr = nc.dram_tensor("bvr", [1, 2 * P], BF16, kind="ExternalInput").ap()
    wo = nc.dram_tensor("wo", [P, 2, D], BF16, kind="ExternalInput").ap()
    zmask = nc.dram_tensor("zmask", [P, 1, P], BF16,
                           kind="ExternalInput").ap()
    yT = nc.dram_tensor("yT", [D, S], BF16, kind="ExternalOutput").ap()

    Exp = mybir.ActivationFunctionType.Exp

    with tile.TileContext(nc) as tc:
        with (
            tc.tile_pool(name="cache", bufs=1) as cache,
            tc.tile_pool(name="e_pool", bufs=8) as e_pool,
            tc.tile_pool(name="c_pool", bufs=8) as c_pool,
            tc.tile_pool(name="r_pool", bufs=2) as r_pool,
            tc.tile_pool(name="y_pool", bufs=3) as y_pool,
            tc.tile_pool(name="sp_ps", bufs=2, space="PSUM") as sp_ps,
            tc.tile_pool(name="av_ps", bufs=2, space="PSUM") as av_ps,
            tc.tile_pool(name="pj_ps", bufs=2, space="PSUM") as pj_ps,
        ):
            def body():
                # ---- persistent caches; ordered and split so the first
                # projection matmul (wq pair 0 + xT chunk-0 c=0) starts
                # ~2us in, with the rest of the stream pipelined behind ----
                wq_sb = cache.tile([P, 2, DCH, P], BF16, tag="wq")
                nc.sync.dma_start(wq_sb[:, 0:1], wq[:, 0:1])
                xT_sb = cache.tile([P, DCH, S], BF16, tag="xT")
                for c in range(DCH):
                    nc.sync.dma_start(xT_sb[:, c:c + 1, ts(0, 512)],
                                      xT[:, c:c + 1, ts(0, 512)])
                bqk_sb = cache.tile([P, 2, 2], F32, tag="bqk")
                nc.sync.dma_start(bqk_sb[:], bqk[:])
                wk_sb = cache.tile([P, 2, DCH, P], BF16, tag="wk")
                nc.sync.dma_start(wk_sb[:, 0:1], wk[:, 0:1])
                nc.sync.dma_start(wq_sb[:, 1:2], wq[:, 1:2])
                nc.sync.dma_start(wk_sb[:, 1:2], wk[:, 1:2])
                wv_sb = cache.tile([P, DCH, 2 * P], BF16, tag="wv")
                nc.sync.dma_start(wv_sb[:], wv[:])
                bvr_sb = cache.tile([P, 2 * P], BF16, tag="bvr")
                nc.sync.dma_start(bvr_sb[0:1, :], bvr[:])
                nc.sync.dma_start(xT_sb[:, :, ts(1, 512)],
                                  xT[:, :, ts(1, 512)])
                z_sb = cache.tile([P, 1, P], BF16, tag="z")
                nc.sync.dma_start(z_sb[:], zmask[:])
                nc.sync.dma_start(xT_sb[:, :, ts(2, 512)],
                                  xT[:, :, ts(2, 512)])
                wo_sb = cache.tile([P, 2, D], BF16, tag="wo")
                nc.sync.dma_start(wo_sb[:], wo[:])
                nc.sync.dma_start(xT_sb[:, :, ts(3, 512)],
                                  xT[:, :, ts(3, 512)])

                qT_sb = cache.tile([P, 2, S], BF16, tag="qT")
                kT_sb = cache.tile([P, 2, S], BF16, tag="kT")
                # per (j-tile, pair): [V_even | ones | V_odd] x 64 cols
                vones = cache.tile([P, NJ, 2, 192], BF16, tag="vones")
                nc.vector.memset(vones[:, :, :, 64:128], 1.0)
                # ones row for the K=1 bias matmul in v_tile
                ones_row = cache.tile([P, P], BF16, tag="ones1")
                nc.vector.memset(ones_row[0:1, :], 1.0)

                if "noexp" in KABL:
                    # ablation only: es is never written, keep reads finite
                    for i in range(8):
                        e0 = e_pool.tile([P, 2, 512], BF16, tag="e",
                                         name=f"einit{i}")
                        nc.vector.memset(e0[:], 0.0)

                # ---- work-unit emitters ----
                def qk_group(t, pr, w_sb, dstT, bcol):
                    ps = pj_ps.tile([P, 512], F32, tag="pj",
                                    name=f"qk{t}{pr}{bcol}")
                    for c in range(DCH):
                        nc.tensor.matmul(
                            ps[:], w_sb[:, pr, c, :],
                            xT_sb[:, c, ts(t, 512)],
                            start=(c == 0), stop=(c == DCH - 1))
                    nc.vector.tensor_scalar_add(
                        dstT[:, pr, ts(t, 512)], ps[:],
                        bqk_sb[:, pr, bcol:bcol + 1])

                def v_tile(jt):
                    # V computed token-major directly (no DMA transposes):
                    # out[tok, chan] = xT_tile.T @ wv, chan order
                    # [Ve0|Ve1|Vo0|Vo1]; bias rides as a K=1 ones matmul
                    ps = pj_ps.tile([P, 8, 64], F32, tag="pj",
                                    name=f"v{jt}")
                    for c in range(DCH):
                        nc.tensor.matmul(
                            ps[:, 0:4, :], xT_sb[:, c, ts(jt, P)],
                            wv_sb[:, c, :], start=(c == 0), stop=False)
                    nc.tensor.matmul(
                        ps[:, 0:4, :], ones_row[0:1, :], bvr_sb[0:1, :],
                        start=False, stop=True)
                    # [Ve0|Ve1] -> even slots, [Vo0|Vo1] -> odd slots
                    nc.scalar.copy(vones[:, jt, :, 0:64], ps[:, 0:2, :])
                    nc.vector.tensor_copy(vones[:, jt, :, 128:192],
                                          ps[:, 2:4, :])

                def proj_fillers(t):
                    fs = []
                    for pr in range(2):
                        for (w_sb, dstT, bcol) in ((wq_sb, qT_sb, 0),
                                                   (wk_sb, kT_sb, 1)):
                            fs.append(lambda t=t, pr=pr, w_sb=w_sb,
                                      dstT=dstT, bcol=bcol:
                                      qk_group(t, pr, w_sb, dstT, bcol))
                    for r in range(4):
                        fs.append(lambda jt=4 * t + r: v_tile(jt))
                    return fs

                def y_group(t, dt_, cc, y_big):
                    # the last chunk's groups run after attention is done, so
                    # they can borrow the idle scores-PSUM slots: 4 banks in
                    # flight instead of 2 keeps the PSUM-evacuation copies
                    # off the matmul critical path
                    last = (t == NT - 1)
                    pool, tag = ((sp_ps, "sp") if last and dt_ % 2 else
                                 (pj_ps, "pj"))
                    yps = pool.tile([P, 512], F32, tag=tag,
                                    name=f"y{t}{dt_}")
                    nc.tensor.matmul(yps[:], wo_sb[:, 0, ts(dt_, P)],
                                     cc[0][:], start=True, stop=False)
                    nc.tensor.matmul(yps[:], wo_sb[:, 1, ts(dt_, P)],
                                     cc[1][:], start=False, stop=True)
                    if "noyT" not in KABL:
                        if last and dt_ % 2 == 0:
                            nc.scalar.copy(y_big[:, dt_, :], yps[:])
                        else:
                            nc.vector.tensor_copy(y_big[:, dt_, :], yps[:])
                        if last and dt_ % 2 == 1:
                            # stream 256-row slabs out as they land so the
                            # epilogue DMA isn't one exposed 1MB burst
                            nc.sync.dma_start(
                                yT[ds(P * (dt_ - 1), 2 * P),
                                   ts(t, 512)].rearrange(
                                    "(c p) s -> p c s", p=P),
                                y_big[:, ds(dt_ - 1, 2), :])
                        elif not last and dt_ == DCH - 1:
                            nc.sync.dma_start(
                                yT[:, ts(t, 512)].rearrange(
                                    "(c p) s -> p c s", p=P), y_big[:])

                def phase3_fillers(t, cc):
                    y_big = y_pool.tile([P, DCH, 512], BF16, tag="y",
                                        name=f"ybig{t}")
                    return [lambda t=t, dt_=dt_, cc=cc, y_big=y_big:
                            y_group(t, dt_, cc, y_big)
                            for dt_ in range(DCH)]

                def attn_visit(t, pr, jt, njt, av):
                    r = jt - 4 * t
                    f0 = P * r if r > 0 else 0
                    sp = sp_ps.tile([P, 2, 512], F32, tag="sp",
                                    name=f"sp{t}{pr}{jt}")
                    for hi in range(2):
                        # timing probe: dupscore forces both head-halves into
                        # PE rows 0:64 so the pair cannot dual-issue
                        hr = 0 if "dupscore" in KABL else hi
                        nc.tensor.matmul(
                            sp[:, hi, f0:512],
                            kT_sb[ds(64 * hr, 64), pr, ts(jt, P)],
                            qT_sb[ds(64 * hr, 64), pr,
                                  ds(512 * t + f0, 512 - f0)],
                            start=True, stop=True)
                    es = e_pool.tile([P, 2, 512], BF16, tag="e",
                                     name=f"es{t}{pr}{jt}")
                    if "noexp" not in KABL:
                        nc.scalar.activation(
                            es[:, :, f0:512], sp[:, :, f0:512],
                            Exp, scale=EXP_SCALE)
                    if r >= 0 and "nomask" not in KABL \
                            and "noexp" not in KABL:
                        # intra-tile causal triangle on the 128 diagonal cols
                        nc.vector.tensor_mul(
                            es[:, :, f0:f0 + P], es[:, :, f0:f0 + P],
                            z_sb[:, :, :].to_broadcast((P, 2, P)))
                    for hi in range(2):
                        # lhsT [V_even | ones] or [ones | V_odd]; only the
                        # i >= f0 columns attend to this j-tile, so the
                        # stale region of es is never read
                        nc.tensor.matmul(
                            av[hi][:, f0:512],
                            vones[:, jt, pr, ds(64 * hi, P)],
                            es[:, hi, f0:512],
                            start=(jt == 0), stop=(jt == njt - 1))

                def normalize(t, pr, av):
                    # av[0]=[out_e|den_e], av[1]=[den_o|out_o].
                    # cross-partition moves must be tensor_copy (DVE
                    # compute ops are lane-locked); 2 crossings is the
                    # minimum since out_h/den_h sit on opposite halves.
                    if "nonorm" in KABL:
                        concat = c_pool.tile([P, 512], BF16, tag="cc",
                                             name=f"cc{t}{pr}")
                        nc.vector.tensor_copy(concat[0:64, :],
                                              av[0][0:64, :])
                        nc.vector.tensor_copy(concat[64:P, :],
                                              av[1][64:P, :])
                        return concat
                    w = r_pool.tile([P, 512], F32, tag="w", name=f"w{t}{pr}")
                    concat = c_pool.tile([P, 512], BF16, tag="cc",
                                         name=f"cc{t}{pr}")
                    if t == NT - 1 and pr == 1:
                        # exposed tail: shorten the serial chain.  Flip
                        # copies run ACT||DVE; the out halves are staged to
                        # SBUF bf16 on ACT (hidden under the reciprocal) so
                        # the muls hit the DVE 2x bf16 path.
                        nc.scalar.copy(w[0:64, :], av[0][64:P, :])
                        nc.vector.tensor_copy(w[64:P, :], av[1][0:64, :])
                        ob = c_pool.tile([P, 512], BF16, tag="cc",
                                         name=f"ob{t}{pr}")
                        nc.scalar.copy(ob[0:64, :], av[0][0:64, :])
                        nc.scalar.copy(ob[64:P, :], av[1][64:P, :])
                        rc = r_pool.tile([P, 512], F32, tag="rc",
                                         name=f"rc{t}{pr}")
                        nc.vector.reciprocal_approx_fast(rc[:], w[:])
                        nc.vector.tensor_mul(concat[:], ob[:], rc[:])
                        return concat
                    nc.vector.tensor_copy(w[0:64, :], av[0][64:P, :])
                    nc.vector.tensor_copy(w[64:P, :], av[1][0:64, :])
                    rc = r_pool.tile([P, 512], F32, tag="rc",
                                     name=f"rc{t}{pr}")
                    nc.vector.reciprocal_approx_fast(rc[:], w[:])
                    nc.vector.tensor_mul(
                        concat[0:64, :], av[0][0:64, :], rc[0:64, :])
                    nc.vector.tensor_mul(
                        concat[64:P, :], av[1][64:P, :], rc[64:P, :])
                    return concat

                # chunk-0 projections up front, then per chunk t: attention
                # visits with filler matmul groups interleaved so the PE has
                # work while ACT runs exp.  The output projections are all
                # deferred to the last chunk: chunks 0-2 are PE-bound on
                # projections anyway, while the last chunk (16 j-tiles of
                # exp, no proj left) is ACT-bound and needs the PE filler.
                for f in proj_fillers(0):
                    f()
                concats = {}
                held = []
                for t in range(NT):
                    fillers = []
                    if t + 1 < NT:
                        fillers += proj_fillers(t + 1)
                    else:
                        for u in range(NT - 1):
                            fillers += phase3_fillers(u, concats[u])
                        # hold a few groups back: they are the only PE work
                        # that can cover the final pair's normalize chain
                        held = fillers[-3:]
                        fillers = fillers[:-3]
                    njt = 4 * t + 4
                    visits = [(pr, jt) for pr in range(2)
                              for jt in range(njt)]
                    nf = len(fillers)
                    nv = len(visits)
                    emitted = 0
                    av = None
                    for vi, (pr, jt) in enumerate(visits):
                        if jt == 0:
                            av = [av_ps.tile([P, 512], F32, tag="av",
                                             name=f"av{t}{pr}{hi}")
                                  for hi in range(2)]
                        attn_visit(t, pr, jt, njt, av)
                        if jt == njt - 1:
                            concats.setdefault(t, {})[pr] = \
                                normalize(t, pr, av)
                        while emitted * nv < (vi + 1) * nf:
                            fillers[emitted]()
                            emitted += 1
                for f in held:
                    f()
                # last chunk's output projection: emit the first four
                # cc[0]-side matmuls up front — they only need pair 0's
                # concat, so they give the PE work to chew on while pair 1's
                # normalize chain runs on ACT/DVE
                cc = concats[NT - 1]
                y_big = y_pool.tile([P, DCH, 512], BF16, tag="y",
                                    name="ybig3")
                ypss = {}

                def y_mm1(dt_):
                    pool, tag = ((sp_ps, "sp") if dt_ % 2 else
                                 (pj_ps, "pj"))
                    yps = pool.tile([P, 512], F32, tag=tag,
                                    name=f"y3{dt_}")
                    nc.tensor.matmul(yps[:], wo_sb[:, 0, ts(dt_, P)],
                                     cc[0][:], start=True, stop=False)
                    ypss[dt_] = yps

                def y_mm2(dt_):
                    yps = ypss[dt_]
                    nc.tensor.matmul(yps[:], wo_sb[:, 1, ts(dt_, P)],
                                     cc[1][:], start=False, stop=True)
                    if "noyT" in KABL:
                        return
                    if dt_ % 2 == 0:
                        nc.scalar.copy(y_big[:, dt_, :], yps[:])
                    else:
                        nc.vector.tensor_copy(y_big[:, dt_, :], yps[:])
                        nc.sync.dma_start(
                            yT[ds(P * (dt_ - 1), 2 * P),
                               ts(NT - 1, 512)].rearrange(
                                "(c p) s -> p c s", p=P),
                            y_big[:, ds(dt_ - 1, 2), :])

                for dt_ in range(4):
                    y_mm1(dt_)
                for dt_ in range(4):
                    y_mm2(dt_)
                for dt_ in range(4, DCH):
                    y_mm1(dt_)
                    y_mm2(dt_)

            if repeat > 1:
                # PE body spans multiple IRAM blocks: hint the back-edge so
                # the branch target is prefetched instead of a ~4us I$ miss
                hints = {
                    "": (),
                    "pe": (mybir.EngineType.PE,),
                    "all": (mybir.EngineType.PE, mybir.EngineType.DVE,
                            mybir.EngineType.Activation, mybir.EngineType.SP,
                            mybir.EngineType.Pool),
                }[os.environ.get("KHINT", "")]
                with tc.For_i(0, repeat, 1, hint_engines=hints,
                              staggered_reset=bool(os.environ.get("KSTAG"))):
                    body()
            else:
                body()

    nc.compile()
    return nc


def make_in_maps(inputs):
    bf16 = ml_dtypes.bfloat16
    x = np.asarray(inputs["x"], dtype=np.float32)
    Wq = np.asarray(inputs["Wq"], dtype=np.float32)
    bq = np.asarray(inputs["bq"], dtype=np.float32)
    Wk = np.asarray(inputs["Wk"], dtype=np.float32)
    bk = np.asarray(inputs["bk"], dtype=np.float32)
    Wv = np.asarray(inputs["Wv"], dtype=np.float32)
    bv = np.asarray(inputs["bv"], dtype=np.float32)
    Wo = np.asarray(inputs["Wo"], dtype=np.float32)

    # intra-tile causal triangle for the diagonal 128x128 block:
    # partition jj = j-token within tile, col ii = i-token offset
    jj = np.arange(P)[:, None]
    ii = np.arange(P)[None, :]
    z = (jj <= ii).astype(np.float32).reshape(P, 1, P).astype(bf16)

    in_maps = []
    for c in range(N_CORES):
        b = c // 4
        g = c % 4
        heads = list(range(HEADS_PER_CORE * g, HEADS_PER_CORE * (g + 1)))
        # xT_pre[p, ch, s] = x[b][s, 128 ch + p]
        xT = np.ascontiguousarray(
            x[b].T.reshape(DCH, P, S).transpose(1, 0, 2)).astype(bf16)
        # wq_pre[p, r, ch, m] = WqT[r][128 ch + p, m], WqT[r] = [D, 128]
        wq_c = np.ascontiguousarray(np.stack([
            Wq[heads[2 * p:2 * p + 2]].reshape(P, D).T.reshape(DCH, P, P)
            for p in range(2)]).transpose(2, 0, 1, 3)).astype(bf16)
        wk_c = np.ascontiguousarray(np.stack([
            Wk[heads[2 * p:2 * p + 2]].reshape(P, D).T.reshape(DCH, P, P)
            for p in range(2)]).transpose(2, 0, 1, 3)).astype(bf16)
        # wv_pre[p, ch, n] = WvT[128 ch + p, n]; WvT = [D, 256] with column
        # order [Ve0|Ve1|Vo0|Vo1] = heads [0, 2, 1, 3] so the v_tile psum
        # splits into even/odd slots with two contiguous copies
        vorder = [heads[0], heads[2], heads[1], heads[3]]
        wv_c = np.ascontiguousarray(
            Wv[vorder].reshape(2 * P, D).T.reshape(
                DCH, P, 2 * P).transpose(1, 0, 2)).astype(bf16)
        bqk = np.stack([
            bq[heads].reshape(2, P),
            bk[heads].reshape(2, P)])                             # [qk, pr, P]
        bvr_c = bv[vorder].reshape(1, 2 * P).astype(bf16)
        # wo[c, p, d] = Wo[d, 256 g + 128 p + c]
        wo_g = Wo[:, 2 * P * g:2 * P * (g + 1)]                   # [D, 256]
        wo_c = np.ascontiguousarray(
            wo_g.T.reshape(2, P, D).transpose(1, 0, 2)).astype(bf16)
        in_maps.append({
            "xT": xT, "wq": wq_c, "wk": wk_c, "wv": wv_c,
            "bqk": np.ascontiguousarray(bqk.transpose(2, 1, 0)),  # [p, pr, qk]
            "bvr": bvr_c, "wo": wo_c, "zmask": z,
        })
    return in_maps


_cached = {}


def _get_module(repeat: int = 1):
    if repeat not in _cached:
        _cached[repeat] = build_module(repeat)
    return _cached[repeat]


def run_cores(inputs, repeat: int = 1):
    nc = _get_module(repeat)
    in_maps = make_in_maps(inputs)
    res = run_bass_kernel_spmd(nc, in_maps, core_ids=list(range(N_CORES)))
    return res.results


def assemble(results, bo):
    y = np.zeros((B, S, D), dtype=np.float32)
    for c in range(N_CORES):
        y[c // 4] += np.asarray(results[c]["yT"], dtype=np.float32).T
    y += np.asarray(bo, dtype=np.float32)[None, None, :]
    return y


def kernel(**inputs):
    results = run_cores(inputs)
    return assemble(results, inputs["bo"])


# revision 36
# speedup vs baseline: 1.0362x; 1.0362x over previous
"""Trainium2 Bass kernel for causal multi-head attention (AbstractNaiveMHA).

Problem shapes (hardcoded per the harness contract):
  x: [B=2, S=2048, D=1024] f32, mask: [B, S] int32 (all ones)
  Wq/Wk/Wv: [H=16, 64, 1024], bq/bk/bv: [16, 64], Wo: [1024, 1024], bo: [1024]

Sharding over 8 NeuronCores: core c -> batch b = c // 4, head group
g = c % 4 (heads 4g..4g+3).  Each core computes its 4 heads' attention
and a partial output projection through its column slice of Wo; the
host sums the 4 partials per batch and adds bo (the "all-reduce").

v3 design notes (all matmuls bf16, rel tol 2e-2 permits it):
  - t-chunked pipeline: for each 512-token i-chunk t: project q/k/v for
    chunk t, run attention for chunk t against j-tiles 0..4t+3, then the
    output projection for chunk t.  Fillers (proj t+1, out-proj t-1)
    overlap the ACT-engine exp.
  - Input DMA split: weights first, then xT per 512-chunk, so the first
    projection matmuls start ~5us in instead of waiting for the full 4MB.
  - Scores computed transposed (S^T[j,i]) with the two heads of a pair in
    PE row-quadrants 0:63/64:127 (quadrant matmuls dual-issue).
  - V path: vT computed v-major, then dma_start_transpose produces the
    token-major [V_even |ones| V_odd] tiles the AV matmul needs.
  - Causal: diagonal j-tiles restrict scores+exp+AV to i >= 128r, so the
    stale region of the es ring is never read (no pre-zeroing, no
    zero-mask); only the 128-wide intra-tile triangle is masked.
  - Softmax denominator rides free in the AV matmul via ones columns:
    av[0]=[out_e|den_e], av[1]=[den_o|out_o]; normalization uses
    cross-partition copies + reciprocal_approx_fast + aligned muls.
  - proj/out-proj PSUM groups share one double-buffered pool so the
    bias-add / PSUM-evacuation of group g drains under group g+1's
    matmuls.
  - Last chunk's tail: output-proj copies alternate ACT/DVE and the yT
    DMA goes out per 128-row tile to shrink the exposed epilogue.
"""

import os

import numpy as np
import ml_dtypes

import concourse.bass as bass
import concourse.mybir as mybir
import concourse.tile as tile
from concourse import bacc
from concourse.bass import ts, ds
from concourse.bass_utils import run_bass_kernel_spmd

B, S, D = 2, 2048, 1024
H, KD = 16, 64
P = 128
NT = S // 512      # 4 i-chunks of 512
NJ = S // P        # 16 j-tiles of 128
DCH = D // P       # 8 contraction chunks for the projections
N_CORES = 8
HEADS_PER_CORE = 4

F32 = mybir.dt.float32
BF16 = mybir.dt.bfloat16
EXP_SCALE = 1.0 / float(np.sqrt(np.float32(KD)))
KABL = set(os.environ.get("KABL", "").split(","))


def build_module(repeat: int = 1):
    """Build the single-core Bass module (same program on all 8 cores)."""
    nc = bacc.Bacc("TRN2", target_bir_lowering=False, debug=False)

    # all inputs arrive pre-swizzled into the SBUF layout (partition dim
    # first) so every cache DMA is a max-rate contiguous transfer
    xT = nc.dram_tensor("xT", [P, DCH, S], BF16, kind="ExternalInput").ap()
    wq = nc.dram_tensor("wq", [P, 2, DCH, P], BF16,
                        kind="ExternalInput").ap()
    wk = nc.dram_tensor("wk", [P, 2, DCH, P], BF16,
                        kind="ExternalInput").ap()
    wv = nc.dram_tensor("wv", [P, DCH, 2 * P], BF16,
                        kind="ExternalInput").ap()
    bqk = nc.dram_tensor("bqk", [P, 2, 2], F32, kind="ExternalInput").ap()
    bvr = nc.dram_tensor("bvr", [1, 2 * P], BF16, kind="ExternalInput").ap()
    wo = nc.dram_tensor("wo", [P, 2, D], BF16, kind="ExternalInput").ap()
    zmask = nc.dram_tensor("zmask", [P, 1, P], BF16,
                           kind="ExternalInput").ap()
    yT = nc.dram_tensor("yT", [D, S], BF16, kind="ExternalOutput").ap()

    Exp = mybir.ActivationFunctionType.Exp

    with tile.TileContext(nc) as tc:
        with (
            tc.tile_pool(name="cache", bufs=1) as cache,
            tc.tile_pool(name="e_pool", bufs=8) as e_pool,
            tc.tile_pool(name="c_pool", bufs=8) as c_pool,
            tc.tile_pool(name="r_pool", bufs=2) as r_pool,
            tc.tile_pool(name="y_pool", bufs=3) as y_pool,
            tc.tile_pool(name="sp_ps", bufs=2, space="PSUM") as sp_ps,
            tc.tile_pool(name="av_ps", bufs=2, space="PSUM") as av_ps,
            tc.tile_pool(name="pj_ps", bufs=2, space="PSUM") as pj_ps,
        ):
            def body():
                # ---- persistent caches; ordered and split so the first
                # projection matmul (wq pair 0 + xT chunk-0 c=0) starts
                # ~2us in, with the rest of the stream pipelined behind ----
                wq_sb = cache.tile([P, 2, DCH, P], BF16, tag="wq")
                nc.sync.dma_start(wq_sb[:, 0:1, 0:1], wq[:, 0:1, 0:1])
                xT_sb = cache.tile([P, DCH, S], BF16, tag="xT")
                nc.sync.dma_start(xT_sb[:, 0:1, ts(0, 512)],
                                  xT[:, 0:1, ts(0, 512)])
                nc.sync.dma_start(wq_sb[:, 0:1, 1:DCH], wq[:, 0:1, 1:DCH])
                nc.sync.dma_start(xT_sb[:, 1:4, ts(0, 512)],
                                  xT[:, 1:4, ts(0, 512)])
                wk_sb = cache.tile([P, 2, DCH, P], BF16, tag="wk")
                nc.sync.dma_start(wk_sb[:, 0:1], wk[:, 0:1])
                nc.sync.dma_start(xT_sb[:, 4:DCH, ts(0, 512)],
                                  xT[:, 4:DCH, ts(0, 512)])
                bqk_sb = cache.tile([P, 2, 2], F32, tag="bqk")
                nc.sync.dma_start(bqk_sb[:], bqk[:])
                nc.sync.dma_start(wq_sb[:, 1:2], wq[:, 1:2])
                nc.sync.dma_start(wk_sb[:, 1:2], wk[:, 1:2])
                wv_sb = cache.tile([P, DCH, 2 * P], BF16, tag="wv")
                nc.sync.dma_start(wv_sb[:], wv[:])
                bvr_sb = cache.tile([P, 2 * P], BF16, tag="bvr")
                nc.sync.dma_start(bvr_sb[0:1, :], bvr[:])
                nc.sync.dma_start(xT_sb[:, :, ts(1, 512)],
                                  xT[:, :, ts(1, 512)])
                z_sb = cache.tile([P, 1, P], BF16, tag="z")
                nc.sync.dma_start(z_sb[:], zmask[:])
                nc.sync.dma_start(xT_sb[:, :, ts(2, 512)],
                                  xT[:, :, ts(2, 512)])
                wo_sb = cache.tile([P, 2, D], BF16, tag="wo")
                nc.sync.dma_start(wo_sb[:], wo[:])
                nc.sync.dma_start(xT_sb[:, :, ts(3, 512)],
                                  xT[:, :, ts(3, 512)])

                qT_sb = cache.tile([P, 2, S], BF16, tag="qT")
                kT_sb = cache.tile([P, 2, S], BF16, tag="kT")
                # per (j-tile, pair): [V_even | ones | V_odd] x 64 cols
                vones = cache.tile([P, NJ, 2, 192], BF16, tag="vones")
                nc.vector.memset(vones[:, :, :, 64:128], 1.0)
                # ones row for the K=1 bias matmul in v_tile
                ones_row = cache.tile([P, P], BF16, tag="ones1")
                nc.vector.memset(ones_row[0:1, :], 1.0)

                if "noexp" in KABL:
                    # ablation only: es is never written, keep reads finite
                    for i in range(8):
                        e0 = e_pool.tile([P, 2, 512], BF16, tag="e",
                                         name=f"einit{i}")
                        nc.vector.memset(e0[:], 0.0)

                # ---- work-unit emitters ----
                def qk_group(t, pr, w_sb, dstT, bcol):
                    ps = pj_ps.tile([P, 512], F32, tag="pj",
                                    name=f"qk{t}{pr}{bcol}")
                    for c in range(DCH):
                        nc.tensor.matmul(
                            ps[:], w_sb[:, pr, c, :],
                            xT_sb[:, c, ts(t, 512)],
                            start=(c == 0), stop=(c == DCH - 1))
                    nc.vector.tensor_scalar_add(
                        dstT[:, pr, ts(t, 512)], ps[:],
                        bqk_sb[:, pr, bcol:bcol + 1])

                def v_tile(jt):
                    # V computed token-major directly (no DMA transposes):
                    # out[tok, chan] = xT_tile.T @ wv, chan order
                    # [Ve0|Ve1|Vo0|Vo1]; bias rides as a K=1 ones matmul
                    ps = pj_ps.tile([P, 8, 64], F32, tag="pj",
                                    name=f"v{jt}")
                    for c in range(DCH):
                        nc.tensor.matmul(
                            ps[:, 0:4, :], xT_sb[:, c, ts(jt, P)],
                            wv_sb[:, c, :], start=(c == 0), stop=False)
                    nc.tensor.matmul(
                        ps[:, 0:4, :], ones_row[0:1, :], bvr_sb[0:1, :],
                        start=False, stop=True)
                    # [Ve0|Ve1] -> even slots, [Vo0|Vo1] -> odd slots
                    nc.scalar.copy(vones[:, jt, :, 0:64], ps[:, 0:2, :])
                    nc.vector.tensor_copy(vones[:, jt, :, 128:192],
                                          ps[:, 2:4, :])

                def proj_fillers(t):
                    fs = []
                    for pr in range(2):
                        for (w_sb, dstT, bcol) in ((wq_sb, qT_sb, 0),
                                                   (wk_sb, kT_sb, 1)):
                            fs.append(lambda t=t, pr=pr, w_sb=w_sb,
                                      dstT=dstT, bcol=bcol:
                                      qk_group(t, pr, w_sb, dstT, bcol))
                    for r in range(4):
                        fs.append(lambda jt=4 * t + r: v_tile(jt))
                    return fs

                def y_group(t, dt_, cc, y_big):
                    # the last chunk's groups run after attention is done, so
                    # they can borrow the idle scores-PSUM slots: 4 banks in
                    # flight instead of 2 keeps the PSUM-evacuation copies
                    # off the matmul critical path
                    last = (t == NT - 1)
                    pool, tag = ((sp_ps, "sp") if last and dt_ % 2 else
                                 (pj_ps, "pj"))
                    yps = pool.tile([P, 512], F32, tag=tag,
                                    name=f"y{t}{dt_}")
                    nc.tensor.matmul(yps[:], wo_sb[:, 0, ts(dt_, P)],
                                     cc[0][:], start=True, stop=False)
                    nc.tensor.matmul(yps[:], wo_sb[:, 1, ts(dt_, P)],
                                     cc[1][:], start=False, stop=True)
                    if "noyT" not in KABL:
                        if last and dt_ % 2 == 0:
                            nc.scalar.copy(y_big[:, dt_, :], yps[:])
                        else:
                            nc.vector.tensor_copy(y_big[:, dt_, :], yps[:])
                        if last and dt_ % 2 == 1:
                            # stream 256-row slabs out as they land so the
                            # epilogue DMA isn't one exposed 1MB burst
                            nc.sync.dma_start(
                                yT[ds(P * (dt_ - 1), 2 * P),
                                   ts(t, 512)].rearrange(
                                    "(c p) s -> p c s", p=P),
                                y_big[:, ds(dt_ - 1, 2), :])
                        elif not last and dt_ == DCH - 1:
                            nc.sync.dma_start(
                                yT[:, ts(t, 512)].rearrange(
                                    "(c p) s -> p c s", p=P), y_big[:])

                def phase3_fillers(t, cc):
                    y_big = y_pool.tile([P, DCH, 512], BF16, tag="y",
                                        name=f"ybig{t}")
                    return [lambda t=t, dt_=dt_, cc=cc, y_big=y_big:
                            y_group(t, dt_, cc, y_big)
                            for dt_ in range(DCH)]

                def attn_visit(t, pr, jt, njt, av):
                    r = jt - 4 * t
                    f0 = P * r if r > 0 else 0
                    sp = sp_ps.tile([P, 2, 512], F32, tag="sp",
                                    name=f"sp{t}{pr}{jt}")
                    for hi in range(2):
                        # timing probe: dupscore forces both head-halves into
                        # PE rows 0:64 so the pair cannot dual-issue
                        hr = 0 if "dupscore" in KABL else hi
                        nc.tensor.matmul(
                            sp[:, hi, f0:512],
                            kT_sb[ds(64 * hr, 64), pr, ts(jt, P)],
                            qT_sb[ds(64 * hr, 64), pr,
                                  ds(512 * t + f0, 512 - f0)],
                            start=True, stop=True)
                    es = e_pool.tile([P, 2, 512], BF16, tag="e",
                                     name=f"es{t}{pr}{jt}")
                    if "noexp" not in KABL:
                        nc.scalar.activation(
                            es[:, :, f0:512], sp[:, :, f0:512],
                            Exp, scale=EXP_SCALE)
                    if r >= 0 and "nomask" not in KABL \
                            and "noexp" not in KABL:
                        # intra-tile causal triangle on the 128 diagonal cols
                        nc.vector.tensor_mul(
                            es[:, :, f0:f0 + P], es[:, :, f0:f0 + P],
                            z_sb[:, :, :].to_broadcast((P, 2, P)))
                    for hi in range(2):
                        # lhsT [V_even | ones] or [ones | V_odd]; only the
                        # i >= f0 columns attend to this j-tile, so the
                        # stale region of es is never read
                        nc.tensor.matmul(
                            av[hi][:, f0:512],
                            vones[:, jt, pr, ds(64 * hi, P)],
                            es[:, hi, f0:512],
                            start=(jt == 0), stop=(jt == njt - 1))

                def normalize(t, pr, av):
                    # av[0]=[out_e|den_e], av[1]=[den_o|out_o].
                    # cross-partition moves must be tensor_copy (DVE
                    # compute ops are lane-locked); 2 crossings is the
                    # minimum since out_h/den_h sit on opposite halves.
                    if "nonorm" in KABL:
                        concat = c_pool.tile([P, 512], BF16, tag="cc",
                                             name=f"cc{t}{pr}")
                        nc.vector.tensor_copy(concat[0:64, :],
                                              av[0][0:64, :])
                        nc.vector.tensor_copy(concat[64:P, :],
                                              av[1][64:P, :])
                        return concat
                    w = r_pool.tile([P, 512], F32, tag="w", name=f"w{t}{pr}")
                    concat = c_pool.tile([P, 512], BF16, tag="cc",
                                         name=f"cc{t}{pr}")
                    if t == NT - 1 and pr == 1:
                        # exposed tail: shorten the serial chain.  Flip
                        # copies run ACT||DVE; the out halves are staged to
                        # SBUF bf16 on ACT (hidden under the reciprocal) so
                        # the muls hit the DVE 2x bf16 path.
                        nc.scalar.copy(w[0:64, :], av[0][64:P, :])
                        nc.vector.tensor_copy(w[64:P, :], av[1][0:64, :])
                        ob = c_pool.tile([P, 512], BF16, tag="cc",
                                         name=f"ob{t}{pr}")
                        nc.scalar.copy(ob[0:64, :], av[0][0:64, :])
                        nc.scalar.copy(ob[64:P, :], av[1][64:P, :])
                        rc = r_pool.tile([P, 512], F32, tag="rc",
                                         name=f"rc{t}{pr}")
                        nc.vector.reciprocal_approx_fast(rc[:], w[:])
                        nc.vector.tensor_mul(concat[:], ob[:], rc[:])
                        return concat
                    nc.vector.tensor_copy(w[0:64, :], av[0][64:P, :])
                    nc.vector.tensor_copy(w[64:P, :], av[1][0:64, :])
                    rc = r_pool.tile([P, 512], F32, tag="rc",
                                     name=f"rc{t}{pr}")
                    nc.vector.reciprocal_approx_fast(rc[:], w[:])
                    nc.vector.tensor_mul(
                        concat[0:64, :], av[0][0:64, :], rc[0:64, :])
                    nc.vector.tensor_mul(
                        concat[64:P, :], av[1][64:P, :], rc[64:P, :])
                    return concat

                # chunk-0 projections up front, then per chunk t: attention
                # visits with filler matmul groups interleaved so the PE has
                # work while ACT runs exp.  The output projections are all
                # deferred to the last chunk: chunks 0-2 are PE-bound on
                # projections anyway, while the last chunk (16 j-tiles of
                # exp, no proj left) is ACT-bound and needs the PE filler.
                for f in proj_fillers(0):
                    f()
                concats = {}
                held = []
                for t in range(NT):
                    fillers = []
                    if t + 1 < NT:
                        fillers += proj_fillers(t + 1)
                    else:
                        for u in range(NT - 1):
                            fillers += phase3_fillers(u, concats[u])
                        # hold a few groups back: they are the only PE work
                        # that can cover the final pair's normalize chain
                        held = fillers[-3:]
                        fillers = fillers[:-3]
                    njt = 4 * t + 4
                    visits = [(pr, jt) for pr in range(2)
                              for jt in range(njt)]
                    nf = len(fillers)
                    nv = len(visits)
                    emitted = 0
                    av = None
                    for vi, (pr, jt) in enumerate(visits):
                        if jt == 0:
                            av = [av_ps.tile([P, 512], F32, tag="av",
                                             name=f"av{t}{pr}{hi}")
                                  for hi in range(2)]
                        attn_visit(t, pr, jt, njt, av)
                        if jt == njt - 1:
                            concats.setdefault(t, {})[pr] = \
                                normalize(t, pr, av)
                        while emitted * nv < (vi + 1) * nf:
                            fillers[emitted]()
                            emitted += 1
                for f in held:
                    f()
                # last chunk's output projection: emit the first four
                # cc[0]-side matmuls up front — they only need pair 0's
                # concat, so they give the PE work to chew on while pair 1's
                # normalize chain runs on ACT/DVE
                cc = concats[NT - 1]
                y_big = y_pool.tile([P, DCH, 512], BF16, tag="y",
                                    name="ybig3")
                ypss = {}

                def y_mm1(dt_):
                    pool, tag = ((sp_ps, "sp") if dt_ % 2 else
                                 (pj_ps, "pj"))
                    yps = pool.tile([P, 512], F32, tag=tag,
                                    name=f"y3{dt_}")
                    nc.tensor.matmul(yps[:], wo_sb[:, 0, ts(dt_, P)],
                                     cc[0][:], start=True, stop=False)
                    ypss[dt_] = yps

                def y_mm2(dt_):
                    yps = ypss[dt_]
                    nc.tensor.matmul(yps[:], wo_sb[:, 1, ts(dt_, P)],
                                     cc[1][:], start=False, stop=True)
                    if "noyT" in KABL:
                        return
                    if dt_ % 2 == 0:
                        nc.scalar.copy(y_big[:, dt_, :], yps[:])
                    else:
                        nc.vector.tensor_copy(y_big[:, dt_, :], yps[:])
                        nc.sync.dma_start(
                            yT[ds(P * (dt_ - 1), 2 * P),
                               ts(NT - 1, 512)].rearrange(
                                "(c p) s -> p c s", p=P),
                            y_big[:, ds(dt_ - 1, 2), :])

                for dt_ in range(4):
                    y_mm1(dt_)
                for dt_ in range(4):
                    y_mm2(dt_)
                for dt_ in range(4, DCH):
                    y_mm1(dt_)
                    y_mm2(dt_)

            if repeat > 1:
                # PE body spans multiple IRAM blocks: hint the back-edge so
                # the branch target is prefetched instead of a ~4us I$ miss
                hints = {
                    "": (),
                    "pe": (mybir.EngineType.PE,),
                    "all": (mybir.EngineType.PE, mybir.EngineType.DVE,
                            mybir.EngineType.Activation, mybir.EngineType.SP,
                            mybir.EngineType.Pool),
                }[os.environ.get("KHINT", "")]
                with tc.For_i(0, repeat, 1, hint_engines=hints,
                              staggered_reset=bool(os.environ.get("KSTAG"))):
                    body()
            else:
                body()

    nc.compile()
    return nc


def make_in_maps(inputs):
    bf16 = ml_dtypes.bfloat16
    x = np.asarray(inputs["x"], dtype=np.float32)
    Wq = np.asarray(inputs["Wq"], dtype=np.float32)
    bq = np.asarray(inputs["bq"], dtype=np.float32)
    Wk = np.asarray(inputs["Wk"], dtype=np.float32)
    bk = np.asarray(inputs["bk"], dtype=np.float32)
    Wv = np.asarray(inputs["Wv"], dtype=np.float32)
    bv = np.asarray(inputs["bv"], dtype=np.float32)
    Wo = np.asarray(inputs["Wo"], dtype=np.float32)

    # intra-tile causal triangle for the diagonal 128x128 block:
    # partition jj = j-token within tile, col ii = i-token offset
    jj = np.arange(P)[:, None]
    ii = np.arange(P)[None, :]
    z = (jj <= ii).astype(np.float32).reshape(P, 1, P).astype(bf16)

    in_maps = []
    for c in range(N_CORES):
        b = c // 4
        g = c % 4
        heads = list(range(HEADS_PER_CORE * g, HEADS_PER_CORE * (g + 1)))
        # xT_pre[p, ch, s] = x[b][s, 128 ch + p]
        xT = np.ascontiguousarray(
            x[b].T.reshape(DCH, P, S).transpose(1, 0, 2)).astype(bf16)
        # wq_pre[p, r, ch, m] = WqT[r][128 ch + p, m], WqT[r] = [D, 128]
        wq_c = np.ascontiguousarray(np.stack([
            Wq[heads[2 * p:2 * p + 2]].reshape(P, D).T.reshape(DCH, P, P)
            for p in range(2)]).transpose(2, 0, 1, 3)).astype(bf16)
        wk_c = np.ascontiguousarray(np.stack([
            Wk[heads[2 * p:2 * p + 2]].reshape(P, D).T.reshape(DCH, P, P)
            for p in range(2)]).transpose(2, 0, 1, 3)).astype(bf16)
        # wv_pre[p, ch, n] = WvT[128 ch + p, n]; WvT = [D, 256] with column
        # order [Ve0|Ve1|Vo0|Vo1] = heads [0, 2, 1, 3] so the v_tile psum
        # splits into even/odd slots with two contiguous copies
        vorder = [heads[0], heads[2], heads[1], heads[3]]
        wv_c = np.ascontiguousarray(
            Wv[vorder].reshape(2 * P, D).T.reshape(
                DCH, P, 2 * P).transpose(1, 0, 2)).astype(bf16)
        bqk = np.stack([
            bq[heads].reshape(2, P),
            bk[heads].reshape(2, P)])                             # [qk, pr, P]
        bvr_c = bv[vorder].reshape(1, 2 * P).astype(bf16)
        # wo[c, p, d] = Wo[d, 256 g + 128 p + c]
        wo_g = Wo[:, 2 * P * g:2 * P * (g + 1)]                   # [D, 256]
        wo_c = np.ascontiguousarray(
            wo_g.T.reshape(2, P, D).transpose(1, 0, 2)).astype(bf16)
        in_maps.append({
            "xT": xT, "wq": wq_c, "wk": wk_c, "wv": wv_c,
            "bqk": np.ascontiguousarray(bqk.transpose(2, 1, 0)),  # [p, pr, qk]
            "bvr": bvr_c, "wo": wo_c, "zmask": z,
        })
    return in_maps


_cached = {}


def _get_module(repeat: int = 1):
    if repeat not in _cached:
        _cached[repeat] = build_module(repeat)
    return _cached[repeat]


def run_cores(inputs, repeat: int = 1):
    nc = _get_module(repeat)
    in_maps = make_in_maps(inputs)
    res = run_bass_kernel_spmd(nc, in_maps, core_ids=list(range(N_CORES)))
    return res.results


def assemble(results, bo):
    y = np.zeros((B, S, D), dtype=np.float32)
    for c in range(N_CORES):
        y[c // 4] += np.asarray(results[c]["yT"], dtype=np.float32).T
    y += np.asarray(bo, dtype=np.float32)[None, None, :]
    return y


def kernel(**inputs):
    results = run_cores(inputs)
    return assemble(results, inputs["bo"])
